# revision 7
# baseline (speedup 1.0000x reference)
"""Trainium2 Bass kernel for nn_Abstract3DBUNet (3D VAE-UNet).

Strategy: every 3x3x3 conv runs on device as tap-wise accumulated fp32r
matmuls (full-rate fp32, fp32 PSUM accumulation), SPMD across 8
NeuronCores sharded batch(2) x D-slab(4). The host prepares zero-padded
(and for small-Cin layers, D-tap-packed) input slabs per core, applies
GroupNorm / pool / upsample / concat / latent sampling between
launches, and reassembles full tensors after each conv launch.

Set BUNET_PROFILE=1 to capture per-launch NTFF profiles; summed
exec_time_ns lands in kernel._EXEC_NS.
"""

import os
import numpy as np
import ml_dtypes

import concourse.bacc as bacc
import concourse.tile as tile
import concourse.mybir as mybir
from concourse.bass_utils import run_bass_kernel_spmd

BF16 = np.dtype(ml_dtypes.bfloat16)
CORE_IDS = list(range(8))
NSLAB = 4  # D-dim slabs per sample; 2 samples x 4 slabs = 8 cores

# ---------------------------------------------------------------- layer cfgs
# mode: 'plain' (27 taps, 3 d-slices), 'trip' (9 taps, K=3*Cin, host packs
# d-shifts into partitions), 'im2col' (Cin=1: K=27, 1 tap)
LAYERS = {
    # name: (Cin, Cout, D, H, W, mode)
    "L0c1": (1, 16, 72, 72, 48, "im2col"),
    "L0c2": (16, 32, 72, 72, 48, "trip"),
    "L1c1": (32, 32, 36, 36, 24, "trip"),
    "L1c2": (32, 64, 36, 36, 24, "trip"),
    "L2c1": (64, 64, 18, 18, 12, "plain"),
    "L2c2": (64, 128, 18, 18, 12, "plain"),
    "L3c1": (128, 128, 9, 9, 6, "plain"),
    "L3c2": (128, 256, 9, 9, 6, "plain"),
    "D0c1": (384, 128, 18, 18, 12, "plain"),
    "D0c2": (128, 128, 18, 18, 12, "plain"),
    "D1c1": (192, 64, 36, 36, 24, "plain"),
    "D1c2": (64, 64, 36, 36, 24, "plain"),
    "D2c1": (96, 32, 72, 72, 48, "plain"),
    "D2c2": (32, 32, 72, 72, 48, "trip"),
}

_TH = {72: 9, 36: 18, 18: 18, 9: 9}  # h-tile rows per matmul (N = Th*W <= 512)


def _plan(name):
    cin, cout, D, H, W, mode = LAYERS[name]
    Ds = -(-D // NSLAB)  # output d-rows per core (ceil)
    if mode == "im2col":
        kmax, nchunks, ndread, ntaps = 27, 1, 1, 1
        rows, rowlen = H, W
    elif mode == "trip":
        kmax, nchunks, ndread, ntaps = 3 * cin, 1, 1, 9
        rows, rowlen = H + 2, W + 2
    else:
        nchunks = -(-cin // 128)
        kmax = min(cin, 128)
        ndread, ntaps = 3, 27
        rows, rowlen = H + 2, W + 2
    ds_in = Ds + (2 if mode == "plain" else 0)
    n_m = -(-cout // 128)
    cout_m = min(cout, 128)
    th = _TH[H]
    return dict(name=name, cin=cin, cout=cout, D=D, H=H, W=W, mode=mode,
                Ds=Ds, kmax=kmax, nchunks=nchunks, ndread=ndread,
                ntaps=ntaps, rows=rows, rowlen=rowlen, ds_in=ds_in,
                n_m=n_m, cout_m=cout_m, th=th)


def _entries(p):
    """Accumulation entries: list of (chunk, kd, kh, kw)."""
    if p["mode"] == "im2col":
        return [(0, 0, 0, 0)]
    if p["mode"] == "trip":
        return [(0, 0, kh, kw) for kh in range(3) for kw in range(3)]
    return [(c, kd, kh, kw) for c in range(p["nchunks"])
            for kd in range(3) for kh in range(3) for kw in range(3)]


_PROGRAMS = {}


def _build_program(name):
    if name in _PROGRAMS:
        return _PROGRAMS[name]
    p = _plan(name)
    ent = _entries(p)
    ne = len(ent)
    nc = bacc.Bacc("TRN2", target_bir_lowering=False, debug=False,
                   num_devices=8)
    inp = nc.dram_tensor("inp", [p["nchunks"], p["ds_in"], p["kmax"],
                                 p["rows"] * p["rowlen"]],
                         mybir.dt.float32, kind="ExternalInput")
    wts = nc.dram_tensor("wts", [p["kmax"], p["n_m"] * ne * p["cout_m"]],
                         mybir.dt.float32, kind="ExternalInput")
    out = nc.dram_tensor("out", [p["Ds"], p["n_m"] * p["cout_m"],
                                 p["H"] * p["W"]],
                         mybir.dt.float32, kind="ExternalOutput")

    W, H, th = p["W"], p["H"], p["th"]
    n_ht = H // th
    with tile.TileContext(nc) as tc:
        with tc.tile_pool(name="wp", bufs=1) as wp, \
             tc.tile_pool(name="ip", bufs=max(2, p["nchunks"] * (p["ndread"] + 2))) as ip, \
             tc.tile_pool(name="op", bufs=4) as op, \
             tc.tile_pool(name="pp", bufs=8, space="PSUM") as pp:
            wstage = wp.tile([p["kmax"], p["n_m"] * ne * p["cout_m"]],
                             mybir.dt.float32, tag="wst")
            nc.sync.dma_start(out=wstage[:], in_=wts[:])
            wt = wp.tile([p["kmax"], p["n_m"] * ne * p["cout_m"]],
                         mybir.dt.float32r, tag="w")
            nc.vector.tensor_copy(wt[:], wstage[:])
            slices = {}
            for d in range(p["Ds"]):
                for c in range(p["nchunks"]):
                    for dd in range(d, d + p["ndread"]):
                        if (c, dd) in slices or dd >= p["ds_in"]:
                            continue
                        st = ip.tile([p["kmax"], p["rows"], p["rowlen"]],
                                     mybir.dt.float32, tag="inst")
                        nc.sync.dma_start(
                            out=st[:],
                            in_=inp[c, dd].rearrange("k (r l) -> k r l",
                                                     r=p["rows"]))
                        t = ip.tile([p["kmax"], p["rows"], p["rowlen"]],
                                    mybir.dt.float32r, tag="in")
                        nc.vector.tensor_copy(t[:], st[:])
                        slices[(c, dd)] = t
                for m in range(p["n_m"]):
                    for ht in range(n_ht):
                        N = th * W
                        ps = pp.tile([p["cout_m"], N], mybir.dt.float32,
                                     tag="ps")
                        for i, (c, kd, kh, kw) in enumerate(ent):
                            t = slices[(c, d + kd)]
                            if p["mode"] == "im2col":
                                rhs = t[:, ht * th: ht * th + th, 0:W]
                            else:
                                rhs = t[:, ht * th + kh: ht * th + kh + th,
                                        kw: kw + W]
                            nc.tensor.matmul(
                                ps[:], wt[:, (m * ne + i) * p["cout_m"]:
                                          (m * ne + i + 1) * p["cout_m"]],
                                rhs, start=(i == 0), stop=(i == ne - 1))
                        ot = op.tile([p["cout_m"], N], mybir.dt.float32,
                                     tag="o")
                        nc.scalar.activation(ot[:], ps[:],
                                             mybir.ActivationFunctionType.Relu)
                        nc.sync.dma_start(
                            out=out[d, m * p["cout_m"]:(m + 1) * p["cout_m"],
                                    ht * N:(ht + 1) * N],
                            in_=ot[:])
    nc.compile()
    _PROGRAMS[name] = (nc, p, ent)
    return _PROGRAMS[name]


# ---------------------------------------------------------------- host glue

def _group_norm(x, g, b, eps=1e-5):
    B, C = x.shape[:2]
    G = 8 if C % 8 == 0 else 1
    xr = x.reshape(B, G, C // G, *x.shape[2:])
    m = xr.mean(axis=(2, 3, 4, 5), keepdims=True, dtype=np.float32)
    v = xr.var(axis=(2, 3, 4, 5), keepdims=True, dtype=np.float32)
    xr = (xr - m) / np.sqrt(v + eps)
    x = xr.reshape(B, C, *x.shape[2:]).astype(np.float32)
    return x * g[None, :, None, None, None] + b[None, :, None, None, None]


def _prep_inputs(p, xn):
    """xn: normalized input (B, Cin, D, H, W) fp32 -> per-core input arrays."""
    B, cin, D, H, W = xn.shape
    Ds, kmax = p["Ds"], p["kmax"]
    xb = xn
    if p["mode"] == "plain":
        pad_c = p["nchunks"] * kmax - cin
        ap = np.pad(xb, ((0, 0), (0, pad_c), (1, 1), (1, 1), (1, 1)))
        # (B, nchunks, kmax, D+2, Hp, Wp) -> slab per core
        ap = ap.reshape(B, p["nchunks"], kmax, D + 2, p["rows"], p["rowlen"])
        full = np.transpose(ap, (0, 1, 3, 2, 4, 5))  # B,nc,D+2,k,rows,rowlen
        src_d, need = D + 2, p["ds_in"]
    elif p["mode"] == "trip":
        ap = np.pad(xb, ((0, 0), (0, 0), (1, 1), (1, 1), (1, 1)))
        trip = np.concatenate([ap[:, :, g:g + D] for g in range(3)], axis=1)
        full = np.transpose(
            trip.reshape(B, 1, kmax, D, p["rows"], p["rowlen"]),
            (0, 1, 3, 2, 4, 5))
        src_d, need = D, Ds
    else:  # im2col, cin == 1
        ap = np.pad(xb, ((0, 0), (0, 0), (1, 1), (1, 1), (1, 1)))
        cols = np.stack([ap[:, 0, kd:kd + D, kh:kh + H, kw:kw + W]
                         for kd in range(3) for kh in range(3)
                         for kw in range(3)], axis=1)  # B,27,D,H,W
        full = np.transpose(
            cols.reshape(B, 1, 27, D, H, W), (0, 1, 3, 2, 4, 5))
        src_d, need = D, Ds
    full = full.reshape(B, p["nchunks"], src_d, kmax,
                        p["rows"] * p["rowlen"])
    arrs = []
    for s in range(B):
        for q in range(NSLAB):
            lo = q * Ds
            sl = full[s, :, lo:lo + need]
            if sl.shape[1] < need:
                sl = np.pad(sl, ((0, 0), (0, need - sl.shape[1]),
                                 (0, 0), (0, 0)))
            arrs.append(np.ascontiguousarray(sl))
    return arrs


def _prep_weights(p, ent, w):
    """w: (Cout, Cin, 3, 3, 3) -> (n_m*ne, kmax, cout_m) bf16."""
    ne = len(ent)
    cout, cin = w.shape[:2]
    W = np.zeros((p["n_m"] * ne, p["kmax"], p["cout_m"]), np.float32)
    for m in range(p["n_m"]):
        olo = m * p["cout_m"]
        ohi = min(cout, olo + p["cout_m"])
        for i, (c, kd, kh, kw) in enumerate(ent):
            if p["mode"] == "im2col":
                for t in range(27):
                    W[m * ne + i, t, :ohi - olo] = \
                        w[olo:ohi, 0, t // 9, (t // 3) % 3, t % 3].T
            elif p["mode"] == "trip":
                for g in range(3):
                    W[m * ne + i, g * cin:(g + 1) * cin, :ohi - olo] = \
                        w[olo:ohi, :, g, kh, kw].T
            else:
                clo = c * p["kmax"]
                chi = min(cin, clo + p["kmax"])
                W[m * ne + i, :chi - clo, :ohi - olo] = \
                    w[olo:ohi, clo:chi, kd, kh, kw].T
    return np.ascontiguousarray(np.transpose(W, (1, 0, 2)).reshape(
        p["kmax"], p["n_m"] * ne * p["cout_m"]))


_EXEC_NS = []  # per-launch neuron-profile exec_time_ns (BUNET_PROFILE=1)
_PROF = None


def _profiling():
    global _PROF
    if _PROF is None:
        _PROF = False
        if os.environ.get("BUNET_PROFILE") == "1":
            try:
                import sys, types
                import antenv
                if "antenv.axon_hooks" not in sys.modules:
                    mod = types.ModuleType("antenv.axon_hooks")
                    _h = [None]
                    mod.set_axon_ntff_profile_hook = \
                        lambda h: _h.__setitem__(0, h)
                    mod.get_axon_ntff_profile_hook = lambda: _h[0]
                    sys.modules["antenv.axon_hooks"] = mod
                    antenv.axon_hooks = mod
                    from trn_agent_boot.trn_boot import \
                        _ntff_profile_via_ctypes
                    mod.set_axon_ntff_profile_hook(
                        _ntff_profile_via_ctypes("/opt/axon/libaxon_pjrt.so"))
                import concourse.bass_utils as bu
                bu.upload_artifacts = lambda d: d  # no S3 here
                _PROF = True
            except Exception:
                _PROF = False
    return _PROF


def _run_conv(name, xn, w):
    """xn: (B,Cin,D,H,W) fp32 normalized input; w: conv weight. Returns
    relu(conv3(xn, w)) as (B,Cout,D,H,W) fp32."""
    nc, p, ent = _build_program(name)
    arrs = _prep_inputs(p, xn)
    warr = _prep_weights(p, ent, np.asarray(w, np.float32))
    in_maps = [{"inp": a, "wts": warr} for a in arrs]
    if _profiling():
        res = run_bass_kernel_spmd(nc, in_maps, core_ids=CORE_IDS,
                                   trace=True)
        if res.exec_time_ns:
            _EXEC_NS.append(res.exec_time_ns)
            print(f"[prof] {name}: {res.exec_time_ns} ns", flush=True)
    else:
        res = run_bass_kernel_spmd(nc, in_maps, core_ids=CORE_IDS)
    B, D, H, W = xn.shape[0], p["D"], p["H"], p["W"]
    out = np.empty((B, p["cout"], D, H, W), np.float32)
    for s in range(B):
        for q in range(NSLAB):
            lo = q * p["Ds"]
            hi = min(D, lo + p["Ds"])
            if hi <= lo:
                continue
            r = res.results[s * NSLAB + q]["out"]  # (Ds, n_m*cout_m, H*W)
            r = np.transpose(r[:hi - lo, :p["cout"]], (1, 0, 2))
            out[s, :, lo:hi] = r.reshape(p["cout"], hi - lo, H, W)
    return out


def _double_conv(tag, x, prm):
    x = _run_conv(tag + "c1", _group_norm(x, np.asarray(prm["g1"], np.float32),
                                          np.asarray(prm["b1"], np.float32)),
                  prm["w1"])
    x = _run_conv(tag + "c2", _group_norm(x, np.asarray(prm["g2"], np.float32),
                                          np.asarray(prm["b2"], np.float32)),
                  prm["w2"])
    return x


def _maxpool2(x):
    B, C, D, H, W = x.shape
    return x.reshape(B, C, D // 2, 2, H // 2, 2, W // 2, 2).max(axis=(3, 5, 7))


def _upsample2(x):
    return x.repeat(2, axis=2).repeat(2, axis=3).repeat(2, axis=4)


def _fix_cov(C):
    def sym_lower(A):
        Al = np.tril(A)
        return (Al + Al.T - np.diag(np.diag(A))).astype(np.float32)
    w, V = np.linalg.eigh(sym_lower(C))
    C1 = (V @ np.diag(w) @ np.linalg.inv(V)).astype(np.float32)
    w2, V2 = np.linalg.eigh(sym_lower(C1))
    neg = w2 < 0
    s = np.float32(neg.sum())
    pos = w2[w2 > 0]
    pmin = np.float32(pos.min()) if pos.size else np.float32(0.0)
    t = s * s * np.float32(100.0) + np.float32(1.0)
    w_new = np.where(neg, pmin * (s - w2) ** 2 / t, w2).astype(np.float32)
    return (V2 @ np.diag(w_new) @ np.linalg.inv(V2)).astype(np.float32)


def kernel(x, eps_latent, enc_params, dec_params, mu_w, mu_b, lv_w, lv_b,
           l2d_w, l2d_b, fin_w, fin_b):
    x = np.asarray(x, np.float32)
    eps_latent = np.asarray(eps_latent, np.float32)
    B = x.shape[0]

    feats = []
    a = x
    for i, prm in enumerate(enc_params):
        if i > 0:
            a = _maxpool2(a)
        a = _double_conv(f"L{i}", a, prm)
        feats.insert(0, a)
    feats = feats[1:]

    flat = a.reshape(B, -1)
    mu = flat @ np.asarray(mu_w, np.float32).T + np.asarray(mu_b, np.float32)
    logvar = (flat @ np.asarray(lv_w, np.float32).T
              + np.asarray(lv_b, np.float32)).reshape(B, 32, 32)
    cov = np.stack([_fix_cov(np.exp(lv / 2)) for lv in logvar])
    chol = np.linalg.cholesky(cov).astype(np.float32)
    sample = mu + np.einsum("bij,bj->bi", chol, eps_latent).astype(np.float32)
    a = (sample @ np.asarray(l2d_w, np.float32).T
         + np.asarray(l2d_b, np.float32)).astype(np.float32)
    a = a.reshape(B, 256, 9, 9, 6)

    for i, (prm, ef) in enumerate(zip(dec_params, feats)):
        a = _upsample2(a)
        a = np.concatenate([ef, a], axis=1)
        a = _double_conv(f"D{i}", a, prm)

    fin_w = np.asarray(fin_w, np.float32)
    fin_b = np.asarray(fin_b, np.float32)
    a = np.einsum("oc,bcdhw->bodhw", fin_w[:, :, 0, 0, 0], a) \
        + fin_b[None, :, None, None, None]
    return (1.0 / (1.0 + np.exp(-a))).astype(np.float32)


# revision 10
# speedup vs baseline: 1.2020x; 1.2020x over previous
"""Trainium2 Bass kernel for nn_Abstract3DBUNet (3D VAE-UNet).

Strategy: every 3x3x3 conv runs on device as tap-wise accumulated fp32r
matmuls (full-rate fp32, fp32 PSUM accumulation), SPMD across 8
NeuronCores sharded batch(2) x D-slab(4). The host prepares zero-padded
(and for small-Cin layers, D-tap-packed) input slabs per core, applies
GroupNorm / pool / upsample / concat / latent sampling between
launches, and reassembles full tensors after each conv launch.

Set BUNET_PROFILE=1 to capture per-launch NTFF profiles; summed
exec_time_ns lands in kernel._EXEC_NS.
"""

import os
import numpy as np
import ml_dtypes

import concourse.bacc as bacc
import concourse.tile as tile
import concourse.mybir as mybir
from concourse.bass_utils import run_bass_kernel_spmd

BF16 = np.dtype(ml_dtypes.bfloat16)
CORE_IDS = list(range(8))
NSLAB = 4  # D-dim slabs per sample; 2 samples x 4 slabs = 8 cores

# ---------------------------------------------------------------- layer cfgs
# mode: 'plain' (27 taps, 3 d-slices), 'trip' (9 taps, K=3*Cin, host packs
# d-shifts into partitions), 'im2col' (Cin=1: K=27, 1 tap)
LAYERS = {
    # name: (Cin, Cout, D, H, W, mode)
    "L0c1": (1, 16, 72, 72, 48, "im2col"),
    "L0c2": (16, 32, 72, 72, 48, "trip"),
    "L1c1": (32, 32, 36, 36, 24, "trip"),
    "L1c2": (32, 64, 36, 36, 24, "trip"),
    "L2c1": (64, 64, 18, 18, 12, "plain"),
    "L2c2": (64, 128, 18, 18, 12, "plain"),
    "L3c1": (128, 128, 9, 9, 6, "plain"),
    "L3c2": (128, 256, 9, 9, 6, "plain"),
    "D0c1": (384, 128, 18, 18, 12, "plain"),
    "D0c2": (128, 128, 18, 18, 12, "plain"),
    "D1c1": (192, 64, 36, 36, 24, "plain"),
    "D1c2": (64, 64, 36, 36, 24, "plain"),
    "D2c1": (96, 32, 72, 72, 48, "plain"),
    "D2c2": (32, 32, 72, 72, 48, "trip"),
}

_TH = {72: 9, 36: 18, 18: 18, 9: 9}  # h-tile rows per matmul (N = Th*W <= 512)


def _plan(name):
    cin, cout, D, H, W, mode = LAYERS[name]
    Ds = -(-D // NSLAB)  # output d-rows per core (ceil)
    if mode == "im2col":
        kmax, nchunks, ndread, ntaps = 27, 1, 1, 1
        rows, rowlen = H, W
    elif mode == "trip":
        kmax, nchunks, ndread, ntaps = 3 * cin, 1, 1, 9
        rows, rowlen = H + 2, W + 2
    else:
        nchunks = -(-cin // 128)
        kmax = min(cin, 128)
        ndread, ntaps = 3, 27
        rows, rowlen = H + 2, W + 2
    ds_in = Ds + (2 if mode == "plain" else 0)
    n_m = -(-cout // 128)
    cout_m = min(cout, 128)
    th = _TH[H]
    return dict(name=name, cin=cin, cout=cout, D=D, H=H, W=W, mode=mode,
                Ds=Ds, kmax=kmax, nchunks=nchunks, ndread=ndread,
                ntaps=ntaps, rows=rows, rowlen=rowlen, ds_in=ds_in,
                n_m=n_m, cout_m=cout_m, th=th)


def _entries(p):
    """Accumulation entries: list of (chunk, kd, kh, kw)."""
    if p["mode"] == "im2col":
        return [(0, 0, 0, 0)]
    if p["mode"] == "trip":
        return [(0, 0, kh, kw) for kh in range(3) for kw in range(3)]
    return [(c, kd, kh, kw) for c in range(p["nchunks"])
            for kd in range(3) for kh in range(3) for kw in range(3)]


_PROGRAMS = {}


def _build_program(name):
    if name in _PROGRAMS:
        return _PROGRAMS[name]
    p = _plan(name)
    ent = _entries(p)
    ne = len(ent)
    nc = bacc.Bacc("TRN2", target_bir_lowering=False, debug=False,
                   num_devices=8)
    inp = nc.dram_tensor("inp", [p["nchunks"], p["ds_in"], p["kmax"],
                                 p["rows"] * p["rowlen"]],
                         mybir.dt.float32, kind="ExternalInput")
    wts = nc.dram_tensor("wts", [p["kmax"], p["n_m"] * ne * p["cout_m"]],
                         mybir.dt.float32, kind="ExternalInput")
    out = nc.dram_tensor("out", [p["Ds"], p["n_m"] * p["cout_m"],
                                 p["H"] * p["W"]],
                         mybir.dt.float32, kind="ExternalOutput")

    W, H, th = p["W"], p["H"], p["th"]
    n_ht = H // th
    db = 1 if th < H or p["mode"] != "plain" else max(1, 256 // (H * W) + 1)
    db = min(db, p["Ds"])  # d-rows per psum tile (plain mode only)
    with tile.TileContext(nc) as tc:
        with tc.tile_pool(name="wp", bufs=1) as wp, \
             tc.tile_pool(name="ip", bufs=(p["nchunks"] if db > 1 else max(2, p["nchunks"] * (p["ndread"] + 2)))) as ip, \
             tc.tile_pool(name="op", bufs=4) as op, \
             tc.tile_pool(name="pp", bufs=8, space="PSUM") as pp:
            wstage = wp.tile([p["kmax"], p["n_m"] * ne * p["cout_m"]],
                             mybir.dt.float32, tag="wst")
            nc.sync.dma_start(out=wstage[:], in_=wts[:])
            wt = wp.tile([p["kmax"], p["n_m"] * ne * p["cout_m"]],
                         mybir.dt.float32r, tag="w")
            nc.vector.tensor_copy(wt[:], wstage[:])
            if db > 1:
                # whole-chunk resident 4D tiles; matmuls span db d-rows
                res4 = []
                for c in range(p["nchunks"]):
                    st = ip.tile([p["kmax"], p["ds_in"],
                                  p["rows"], p["rowlen"]],
                                 mybir.dt.float32, tag="inst")
                    nc.sync.dma_start(
                        out=st[:],
                        in_=inp[c].rearrange("d k (r l) -> k d r l",
                                             r=p["rows"]))
                    t = ip.tile([p["kmax"], p["ds_in"],
                                 p["rows"], p["rowlen"]],
                                mybir.dt.float32r, tag="in")
                    nc.vector.tensor_copy(t[:], st[:])
                    res4.append(t)
                for d in range(0, p["Ds"], db):
                    nd = min(db, p["Ds"] - d)
                    for m in range(p["n_m"]):
                        N = nd * H * W
                        ps = pp.tile([p["cout_m"], N], mybir.dt.float32,
                                     tag="ps")
                        for i, (c, kd, kh, kw) in enumerate(ent):
                            rhs = res4[c][:, d + kd: d + kd + nd,
                                          kh: kh + H, kw: kw + W]
                            nc.tensor.matmul(
                                ps[:], wt[:, (m * ne + i) * p["cout_m"]:
                                          (m * ne + i + 1) * p["cout_m"]],
                                rhs, start=(i == 0), stop=(i == ne - 1))
                        ot = op.tile([p["cout_m"], N], mybir.dt.float32,
                                     tag="o")
                        nc.scalar.activation(ot[:], ps[:],
                                             mybir.ActivationFunctionType.Relu)
                        nc.sync.dma_start(
                            out=out[d: d + nd,
                                    m * p["cout_m"]:(m + 1) * p["cout_m"],
                                    :].rearrange("d c n -> c d n"),
                            in_=ot[:].rearrange("c (d n) -> c d n", d=nd))
            else:
                slices = {}
                for d in range(p["Ds"]):
                    for c in range(p["nchunks"]):
                        for dd in range(d, d + p["ndread"]):
                            if (c, dd) in slices or dd >= p["ds_in"]:
                                continue
                            st = ip.tile([p["kmax"], p["rows"], p["rowlen"]],
                                         mybir.dt.float32, tag="inst")
                            nc.sync.dma_start(
                                out=st[:],
                                in_=inp[c, dd].rearrange("k (r l) -> k r l",
                                                         r=p["rows"]))
                            t = ip.tile([p["kmax"], p["rows"], p["rowlen"]],
                                        mybir.dt.float32r, tag="in")
                            nc.vector.tensor_copy(t[:], st[:])
                            slices[(c, dd)] = t
                    for m in range(p["n_m"]):
                        for ht in range(n_ht):
                            N = th * W
                            ps = pp.tile([p["cout_m"], N], mybir.dt.float32,
                                         tag="ps")
                            for i, (c, kd, kh, kw) in enumerate(ent):
                                t = slices[(c, d + kd)]
                                if p["mode"] == "im2col":
                                    rhs = t[:, ht * th: ht * th + th, 0:W]
                                else:
                                    rhs = t[:, ht * th + kh: ht * th + kh + th,
                                            kw: kw + W]
                                nc.tensor.matmul(
                                    ps[:], wt[:, (m * ne + i) * p["cout_m"]:
                                              (m * ne + i + 1) * p["cout_m"]],
                                    rhs, start=(i == 0), stop=(i == ne - 1))
                            ot = op.tile([p["cout_m"], N], mybir.dt.float32,
                                         tag="o")
                            nc.scalar.activation(
                                ot[:], ps[:],
                                mybir.ActivationFunctionType.Relu)
                            nc.sync.dma_start(
                                out=out[d, m * p["cout_m"]:
                                        (m + 1) * p["cout_m"],
                                        ht * N:(ht + 1) * N],
                                in_=ot[:])
    nc.compile()
    _PROGRAMS[name] = (nc, p, ent)
    return _PROGRAMS[name]


# ---------------------------------------------------------------- host glue

def _group_norm(x, g, b, eps=1e-5):
    B, C = x.shape[:2]
    G = 8 if C % 8 == 0 else 1
    xr = x.reshape(B, G, C // G, *x.shape[2:])
    m = xr.mean(axis=(2, 3, 4, 5), keepdims=True, dtype=np.float32)
    v = xr.var(axis=(2, 3, 4, 5), keepdims=True, dtype=np.float32)
    xr = (xr - m) / np.sqrt(v + eps)
    x = xr.reshape(B, C, *x.shape[2:]).astype(np.float32)
    return x * g[None, :, None, None, None] + b[None, :, None, None, None]


def _prep_inputs(p, xn):
    """xn: normalized input (B, Cin, D, H, W) fp32 -> per-core input arrays."""
    B, cin, D, H, W = xn.shape
    Ds, kmax = p["Ds"], p["kmax"]
    xb = xn
    if p["mode"] == "plain":
        pad_c = p["nchunks"] * kmax - cin
        ap = np.pad(xb, ((0, 0), (0, pad_c), (1, 1), (1, 1), (1, 1)))
        # (B, nchunks, kmax, D+2, Hp, Wp) -> slab per core
        ap = ap.reshape(B, p["nchunks"], kmax, D + 2, p["rows"], p["rowlen"])
        full = np.transpose(ap, (0, 1, 3, 2, 4, 5))  # B,nc,D+2,k,rows,rowlen
        src_d, need = D + 2, p["ds_in"]
    elif p["mode"] == "trip":
        ap = np.pad(xb, ((0, 0), (0, 0), (1, 1), (1, 1), (1, 1)))
        trip = np.concatenate([ap[:, :, g:g + D] for g in range(3)], axis=1)
        full = np.transpose(
            trip.reshape(B, 1, kmax, D, p["rows"], p["rowlen"]),
            (0, 1, 3, 2, 4, 5))
        src_d, need = D, Ds
    else:  # im2col, cin == 1
        ap = np.pad(xb, ((0, 0), (0, 0), (1, 1), (1, 1), (1, 1)))
        cols = np.stack([ap[:, 0, kd:kd + D, kh:kh + H, kw:kw + W]
                         for kd in range(3) for kh in range(3)
                         for kw in range(3)], axis=1)  # B,27,D,H,W
        full = np.transpose(
            cols.reshape(B, 1, 27, D, H, W), (0, 1, 3, 2, 4, 5))
        src_d, need = D, Ds
    full = full.reshape(B, p["nchunks"], src_d, kmax,
                        p["rows"] * p["rowlen"])
    arrs = []
    for s in range(B):
        for q in range(NSLAB):
            lo = q * Ds
            sl = full[s, :, lo:lo + need]
            if sl.shape[1] < need:
                sl = np.pad(sl, ((0, 0), (0, need - sl.shape[1]),
                                 (0, 0), (0, 0)))
            arrs.append(np.ascontiguousarray(sl))
    return arrs


def _prep_weights(p, ent, w):
    """w: (Cout, Cin, 3, 3, 3) -> (n_m*ne, kmax, cout_m) bf16."""
    ne = len(ent)
    cout, cin = w.shape[:2]
    W = np.zeros((p["n_m"] * ne, p["kmax"], p["cout_m"]), np.float32)
    for m in range(p["n_m"]):
        olo = m * p["cout_m"]
        ohi = min(cout, olo + p["cout_m"])
        for i, (c, kd, kh, kw) in enumerate(ent):
            if p["mode"] == "im2col":
                for t in range(27):
                    W[m * ne + i, t, :ohi - olo] = \
                        w[olo:ohi, 0, t // 9, (t // 3) % 3, t % 3].T
            elif p["mode"] == "trip":
                for g in range(3):
                    W[m * ne + i, g * cin:(g + 1) * cin, :ohi - olo] = \
                        w[olo:ohi, :, g, kh, kw].T
            else:
                clo = c * p["kmax"]
                chi = min(cin, clo + p["kmax"])
                W[m * ne + i, :chi - clo, :ohi - olo] = \
                    w[olo:ohi, clo:chi, kd, kh, kw].T
    return np.ascontiguousarray(np.transpose(W, (1, 0, 2)).reshape(
        p["kmax"], p["n_m"] * ne * p["cout_m"]))


_EXEC_NS = []  # per-launch neuron-profile exec_time_ns (BUNET_PROFILE=1)
_PROF = None


def _profiling():
    global _PROF
    if _PROF is None:
        _PROF = False
        if os.environ.get("BUNET_PROFILE") == "1":
            try:
                import sys, types
                import antenv
                if "antenv.axon_hooks" not in sys.modules:
                    mod = types.ModuleType("antenv.axon_hooks")
                    _h = [None]
                    mod.set_axon_ntff_profile_hook = \
                        lambda h: _h.__setitem__(0, h)
                    mod.get_axon_ntff_profile_hook = lambda: _h[0]
                    sys.modules["antenv.axon_hooks"] = mod
                    antenv.axon_hooks = mod
                    from trn_agent_boot.trn_boot import \
                        _ntff_profile_via_ctypes
                    mod.set_axon_ntff_profile_hook(
                        _ntff_profile_via_ctypes("/opt/axon/libaxon_pjrt.so"))
                import concourse.bass_utils as bu
                bu.upload_artifacts = lambda d: d  # no S3 here
                _PROF = True
            except Exception:
                _PROF = False
    return _PROF


def _run_conv(name, xn, w):
    """xn: (B,Cin,D,H,W) fp32 normalized input; w: conv weight. Returns
    relu(conv3(xn, w)) as (B,Cout,D,H,W) fp32."""
    nc, p, ent = _build_program(name)
    arrs = _prep_inputs(p, xn)
    warr = _prep_weights(p, ent, np.asarray(w, np.float32))
    in_maps = [{"inp": a, "wts": warr} for a in arrs]
    if _profiling():
        res = run_bass_kernel_spmd(nc, in_maps, core_ids=CORE_IDS,
                                   trace=True)
        if res.exec_time_ns:
            _EXEC_NS.append(res.exec_time_ns)
            print(f"[prof] {name}: {res.exec_time_ns} ns", flush=True)
    else:
        res = run_bass_kernel_spmd(nc, in_maps, core_ids=CORE_IDS)
    B, D, H, W = xn.shape[0], p["D"], p["H"], p["W"]
    out = np.empty((B, p["cout"], D, H, W), np.float32)
    for s in range(B):
        for q in range(NSLAB):
            lo = q * p["Ds"]
            hi = min(D, lo + p["Ds"])
            if hi <= lo:
                continue
            r = res.results[s * NSLAB + q]["out"]  # (Ds, n_m*cout_m, H*W)
            r = np.transpose(r[:hi - lo, :p["cout"]], (1, 0, 2))
            out[s, :, lo:hi] = r.reshape(p["cout"], hi - lo, H, W)
    return out


def _double_conv(tag, x, prm):
    x = _run_conv(tag + "c1", _group_norm(x, np.asarray(prm["g1"], np.float32),
                                          np.asarray(prm["b1"], np.float32)),
                  prm["w1"])
    x = _run_conv(tag + "c2", _group_norm(x, np.asarray(prm["g2"], np.float32),
                                          np.asarray(prm["b2"], np.float32)),
                  prm["w2"])
    return x


def _maxpool2(x):
    B, C, D, H, W = x.shape
    return x.reshape(B, C, D // 2, 2, H // 2, 2, W // 2, 2).max(axis=(3, 5, 7))


def _upsample2(x):
    return x.repeat(2, axis=2).repeat(2, axis=3).repeat(2, axis=4)


def _fix_cov(C):
    def sym_lower(A):
        Al = np.tril(A)
        return (Al + Al.T - np.diag(np.diag(A))).astype(np.float32)
    w, V = np.linalg.eigh(sym_lower(C))
    C1 = (V @ np.diag(w) @ np.linalg.inv(V)).astype(np.float32)
    w2, V2 = np.linalg.eigh(sym_lower(C1))
    neg = w2 < 0
    s = np.float32(neg.sum())
    pos = w2[w2 > 0]
    pmin = np.float32(pos.min()) if pos.size else np.float32(0.0)
    t = s * s * np.float32(100.0) + np.float32(1.0)
    w_new = np.where(neg, pmin * (s - w2) ** 2 / t, w2).astype(np.float32)
    return (V2 @ np.diag(w_new) @ np.linalg.inv(V2)).astype(np.float32)


def kernel(x, eps_latent, enc_params, dec_params, mu_w, mu_b, lv_w, lv_b,
           l2d_w, l2d_b, fin_w, fin_b):
    x = np.asarray(x, np.float32)
    eps_latent = np.asarray(eps_latent, np.float32)
    B = x.shape[0]

    feats = []
    a = x
    for i, prm in enumerate(enc_params):
        if i > 0:
            a = _maxpool2(a)
        a = _double_conv(f"L{i}", a, prm)
        feats.insert(0, a)
    feats = feats[1:]

    flat = a.reshape(B, -1)
    mu = flat @ np.asarray(mu_w, np.float32).T + np.asarray(mu_b, np.float32)
    logvar = (flat @ np.asarray(lv_w, np.float32).T
              + np.asarray(lv_b, np.float32)).reshape(B, 32, 32)
    cov = np.stack([_fix_cov(np.exp(lv / 2)) for lv in logvar])
    chol = np.linalg.cholesky(cov).astype(np.float32)
    sample = mu + np.einsum("bij,bj->bi", chol, eps_latent).astype(np.float32)
    a = (sample @ np.asarray(l2d_w, np.float32).T
         + np.asarray(l2d_b, np.float32)).astype(np.float32)
    a = a.reshape(B, 256, 9, 9, 6)

    for i, (prm, ef) in enumerate(zip(dec_params, feats)):
        a = _upsample2(a)
        a = np.concatenate([ef, a], axis=1)
        a = _double_conv(f"D{i}", a, prm)

    fin_w = np.asarray(fin_w, np.float32)
    fin_b = np.asarray(fin_b, np.float32)
    a = np.einsum("oc,bcdhw->bodhw", fin_w[:, :, 0, 0, 0], a) \
        + fin_b[None, :, None, None, None]
    return (1.0 / (1.0 + np.exp(-a))).astype(np.float32)
